# revision 1
# baseline (speedup 1.0000x reference)
"""MoE (top-2 of 8 experts + 1 shared expert, SwiGLU FFN) on 8 TRN2 NeuronCores.

Strategy (expert-parallel, per the sharding hint):
  - Host computes the (tiny) gate: softmax top-2 over E=8 for T=8192 tokens,
    and from it the dispatch: per-expert gathered token lists, ordered by
    owning core, plus scatter/gather index maps. This is the "sharding" step;
    >99.9% of FLOPs (the FFNs) run on device.
  - Core e receives the tokens routed to expert e (transposed, [D, C_cap]),
    runs the SwiGLU FFN in two token-half passes, scales rows by the gate
    weight, and scatters rows into per-half AllToAll dispatch buffers laid
    out by destination core. Each half's AllToAll fires as soon as that
    half's outputs are ready, overlapping the remaining compute.
  - Each core also runs the shared expert on its own T/8 token slice
    (overlapped with the second collective).
  - Combine on device: out[t] = shared(t) + contrib0(t) + contrib1(t).
  - Host concatenates the 8 [T/8, D] output slices. No host math beyond the
    gate.

Compute dtype is fp16 (11-bit relative precision, ~5e-4 dot-product rel err
vs the fp32 reference), which runs the TensorEngine at full rate with hidden
weight loads and halves all DMA traffic. PSUM accumulation stays fp32.
"""
import contextlib

import numpy as np

import concourse.bass as bass
import concourse.tile as tile
from concourse import bacc, mybir
from concourse.bass_utils import run_bass_kernel_spmd

# problem shape (hardcoded per contract)
T = 8192
D = 1024
F = 4096
E = 8
TOPK = 2
NCORES = 8
TO = T // NCORES          # tokens owned per core

F32 = mybir.dt.float32
F16 = mybir.dt.float16
I32 = mybir.dt.int32

# default capacities; bumped (with recompile) if routing demands more
C_CAP_DEFAULT = 2304      # max tokens per expert (pad target, mult of 256)
P_CAP_DEFAULT = 256       # max tokens per (expert, owner, token-half) chunk

_nc_cache: dict[tuple, object] = {}


def _chunk_slices(c_len):
    """Moving-dim chunks of <=512, each >=256 so LDWEIGHTS stays hidden."""
    out = []
    pos = 0
    rem = c_len
    while rem > 0:
        if rem > 512:
            w = 512 if rem - 512 >= 256 else 384
        else:
            w = rem
        out.append((pos, w))
        pos += w
        rem -= w
    return out


def _ffn_phase(nc, sbuf, psum, xk_tiles, w1d, w2d, b1t, c_len, y_tiles):
    """Emit one SwiGLU FFN pass over c_len tokens.

    xk_tiles: [k][ci] SBUF tiles [128, cw] (fp16), contraction-major slices.
    w1d: DRAM [2F/128=64, D/128=8, 128, 128] fp16 (host-tiled, lhsT layout)
    w2d: DRAM [F, D] fp16
    b1t: SBUF [128, 64] f32 (column m = bias for 2F-chunk m)
    y_tiles: c_len//128 SBUF tiles [128, D] f32 receiving the FFN output
             (written on slab 0, accumulated on slabs 1..3).
    """
    n_t = c_len // 128
    KD = D // 128                       # 8 contraction chunks for stage 1
    MF = F // 128                       # 32 f-chunks
    N_SLAB = 4
    per_slab = MF // N_SLAB             # 8 f-chunks per slab
    chunks = _chunk_slices(c_len)

    for q in range(N_SLAB):
        g_tiles = []
        for fi in range(per_slab):
            mp = q * per_slab + fi      # a-chunk index; b-chunk = mp + MF
            w1a = sbuf.tile([128, KD, 128], F16, tag="w1a", name="w1a", bufs=6)
            w1b = sbuf.tile([128, KD, 128], F16, tag="w1b", name="w1b", bufs=6)
            for k in range(KD):
                nc.sync.dma_start(out=w1a[:, k, :], in_=w1d[mp, k])
                nc.sync.dma_start(out=w1b[:, k, :], in_=w1d[mp + MF, k])
            g_t = sbuf.tile([128, c_len], F16, tag=f"g{fi}", name=f"g{fi}",
                            bufs=1)
            for ci, (cs, cw) in enumerate(chunks):
                ps_a = psum.tile([128, 512], F32, space="PSUM", tag="ps_a",
                                 name="ps_a", bufs=3)
                ps_b = psum.tile([128, 512], F32, space="PSUM", tag="ps_b",
                                 name="ps_b", bufs=3)
                for k in range(KD):
                    nc.tensor.matmul(out=ps_a[:, :cw], lhsT=w1a[:, k, :],
                                     rhs=xk_tiles[k][ci][:, :cw],
                                     start=(k == 0), stop=(k == KD - 1))
                for k in range(KD):
                    nc.tensor.matmul(out=ps_b[:, :cw], lhsT=w1b[:, k, :],
                                     rhs=xk_tiles[k][ci][:, :cw],
                                     start=(k == 0), stop=(k == KD - 1))
                t_a = sbuf.tile([128, 512], F16, tag="t_a", name="t_a", bufs=3)
                t_b = sbuf.tile([128, 512], F16, tag="t_b", name="t_b", bufs=3)
                nc.scalar.activation(t_a[:, :cw], ps_a[:, :cw],
                                     mybir.ActivationFunctionType.Silu,
                                     bias=b1t[:, mp:mp + 1])
                nc.scalar.activation(t_b[:, :cw], ps_b[:, :cw],
                                     mybir.ActivationFunctionType.Identity,
                                     bias=b1t[:, mp + MF:mp + MF + 1])
                nc.vector.tensor_mul(g_t[:, cs:cs + cw], t_a[:, :cw],
                                     t_b[:, :cw])
            g_tiles.append(g_t)
            if fi == 0:
                # stage-2 weights for this slab (stream during stage-1)
                w2t = sbuf.tile([128, per_slab, D], F16, tag="w2",
                                name="w2", bufs=1)
                for fj in range(per_slab):
                    f2 = q * per_slab + fj
                    nc.sync.dma_start(out=w2t[:, fj, :],
                                      in_=w2d[f2 * 128:(f2 + 1) * 128, :])
        # stage-2 partial: y (+)= g_slab.T @ w2_slab
        for t in range(n_t):
            ts = slice(t * 128, (t + 1) * 128)
            for d in range(D // 512):
                ds = slice(d * 512, (d + 1) * 512)
                ps_y = psum.tile([128, 512], F32, space="PSUM", tag="ps_y",
                                 name="ps_y", bufs=2)
                for fi in range(per_slab):
                    nc.tensor.matmul(out=ps_y[:],
                                     lhsT=g_tiles[fi][:, ts],
                                     rhs=w2t[:, fi, ds],
                                     start=(fi == 0), stop=(fi == per_slab - 1))
                if q == 0:
                    nc.vector.tensor_copy(y_tiles[t][:, ds], ps_y[:])
                else:
                    nc.vector.tensor_add(y_tiles[t][:, ds], y_tiles[t][:, ds],
                                         ps_y[:])


def _load_x_chunks(nc, sbuf, xdram, col0, c_len, tag_prefix):
    """Load contraction-major x slices as per-chunk tiles [k][ci]."""
    chunks = _chunk_slices(c_len)
    tiles = [[None] * len(chunks) for _ in range(D // 128)]
    for ci, (cs, cw) in enumerate(chunks):
        for k in range(D // 128):
            xt = sbuf.tile([128, 512], F16, tag=f"{tag_prefix}{k}_{ci}",
                           name=f"xc{k}_{ci}", bufs=2)
            nc.sync.dma_start(out=xt[:, :cw],
                              in_=xdram[k][:, col0 + cs:col0 + cs + cw])
            tiles[k][ci] = xt
    return tiles


def _build(c_cap, p_cap):
    key = (c_cap, p_cap)
    if key in _nc_cache:
        return _nc_cache[key]

    nc = bacc.Bacc("TRN2", target_bir_lowering=False, debug=False,
                   num_devices=NCORES)

    def din(name, shape, dt):
        return nc.dram_tensor(name, shape, dt, kind="ExternalInput").ap()

    xg = din("xg", [D // 128, 128, c_cap], F16)        # gathered tokens^T
    xs = din("xs", [D // 128, 128, TO], F16)           # owned tokens^T
    w1 = din("w1", [2 * F // 128, D // 128, 128, 128], F16)
    w2 = din("w2", [F, D], F16)
    sw1 = din("sw1", [2 * F // 128, D // 128, 128, 128], F16)
    sw2 = din("sw2", [F, D], F16)
    b1 = din("b1", [128, 2 * F // 128], F32)           # col m = chunk-m bias
    sb1 = din("sb1", [128, 2 * F // 128], F32)
    b2 = din("b2", [1, D], F32)
    sb2 = din("sb2", [1, D], F32)
    cwd = din("cw", [c_cap, 1], F32)                   # combine weights
    scat = din("scat", [c_cap, 1], I32)                # row in half's a2a_in
    g0i = din("g0i", [TO, 1], I32)                     # abs row in a2a_out
    g1i = din("g1i", [TO, 1], I32)
    out = nc.dram_tensor("out", [TO, D], F32, kind="ExternalOutput").ap()

    c_half = c_cap // 2
    rows_h = NCORES * p_cap                            # rows per half buffer

    with tile.TileContext(nc) as tc:
        with contextlib.ExitStack() as ctx:
            sbuf = ctx.enter_context(tc.tile_pool(name="sbuf", bufs=1))
            psum = ctx.enter_context(tc.tile_pool(name="psum", bufs=2,
                                                  space="PSUM"))
            dpool = ctx.enter_context(tc.tile_pool(name="dram", bufs=1,
                                                   space="DRAM"))

            a2a_in0 = dpool.tile([rows_h, D], F16)
            a2a_in1 = dpool.tile([rows_h, D], F16)
            a2a_out = dpool.tile([2 * rows_h, D], F16)
            a2a_ins = [a2a_in0, a2a_in1]

            # biases (resident)
            b1t = sbuf.tile([128, 2 * F // 128], F32, tag="b1t", name="b1t",
                            bufs=1)
            sb1t = sbuf.tile([128, 2 * F // 128], F32, tag="sb1t",
                             name="sb1t", bufs=1)
            nc.sync.dma_start(out=b1t[:], in_=b1[:])
            nc.sync.dma_start(out=sb1t[:], in_=sb1[:])
            b2t = sbuf.tile([128, D], F32, tag="b2t", name="b2t", bufs=1)
            sb2t = sbuf.tile([128, D], F32, tag="sb2t", name="sb2t", bufs=1)
            nc.gpsimd.dma_start(out=b2t[:], in_=b2.to_broadcast([128, D]))
            nc.gpsimd.dma_start(out=sb2t[:], in_=sb2.to_broadcast([128, D]))

            # ---------------- routed expert (2 half-passes over tokens) ----
            for h in range(2):
                hs = h * c_half
                xk_tiles = _load_x_chunks(nc, sbuf, xg, hs, c_half, "xk")
                y_tiles = [sbuf.tile([128, D], F32, tag=f"ya{t}",
                                     name=f"ya{t}", bufs=1)
                           for t in range(c_half // 128)]

                _ffn_phase(nc, sbuf, psum, xk_tiles, w1, w2, b1t, c_half,
                           y_tiles)
                # finalize: +b2, *combine weight, scatter to this half's buf
                for t in range(c_half // 128):
                    row0 = hs + t * 128
                    cwt = sbuf.tile([128, 1], F32, tag="cwt", name="cwt",
                                    bufs=2)
                    sct = sbuf.tile([128, 1], I32, tag="sct", name="sct",
                                    bufs=2)
                    nc.sync.dma_start(out=cwt[:], in_=cwd[row0:row0 + 128, :])
                    nc.sync.dma_start(out=sct[:], in_=scat[row0:row0 + 128, :])
                    yt = y_tiles[t]
                    yh = sbuf.tile([128, D], F16, tag="yh", name="yh", bufs=2)
                    nc.vector.tensor_add(yt[:], yt[:], b2t[:])
                    nc.vector.tensor_scalar_mul(yh[:], yt[:], cwt[:, :1])
                    nc.gpsimd.indirect_dma_start(
                        out=a2a_ins[h][:],
                        out_offset=bass.IndirectOffsetOnAxis(ap=sct[:, :1],
                                                             axis=0),
                        in_=yh[:],
                        in_offset=None,
                        bounds_check=rows_h - 1,
                        oob_is_err=False,
                    )
                # dispatch this half back to the owner cores
                nc.gpsimd.collective_compute(
                    "AllToAll",
                    mybir.AluOpType.bypass,
                    replica_groups=[list(range(NCORES))],
                    ins=[a2a_ins[h][:].opt()],
                    outs=[a2a_out[h * rows_h:(h + 1) * rows_h, :].opt()],
                )

            # ---------------- shared expert on owned tokens (overlaps) ----
            sk_tiles = _load_x_chunks(nc, sbuf, xs, 0, TO, "xk")
            ys_tiles = [sbuf.tile([128, D], F32, tag=f"ya{t}", name=f"ya{t}",
                                  bufs=1)
                        for t in range(TO // 128)]

            _ffn_phase(nc, sbuf, psum, sk_tiles, sw1, sw2, sb1t, TO,
                       ys_tiles)

            # combine on GpSimd so it overlaps the tail of the shared phase
            for t in range(TO // 128):
                row0 = t * 128
                i0 = sbuf.tile([128, 1], I32, tag="i0", name="i0", bufs=2)
                i1 = sbuf.tile([128, 1], I32, tag="i1", name="i1", bufs=2)
                nc.sync.dma_start(out=i0[:], in_=g0i[row0:row0 + 128, :])
                nc.sync.dma_start(out=i1[:], in_=g1i[row0:row0 + 128, :])
                r0 = sbuf.tile([128, D], F16, tag="r0", name="r0", bufs=2)
                r1 = sbuf.tile([128, D], F16, tag="r1", name="r1", bufs=2)
                nc.gpsimd.indirect_dma_start(
                    out=r0[:], out_offset=None, in_=a2a_out[:],
                    in_offset=bass.IndirectOffsetOnAxis(ap=i0[:, :1], axis=0))
                nc.gpsimd.indirect_dma_start(
                    out=r1[:], out_offset=None, in_=a2a_out[:],
                    in_offset=bass.IndirectOffsetOnAxis(ap=i1[:, :1], axis=0))
                yt = ys_tiles[t]
                nc.vector.tensor_add(yt[:], yt[:], sb2t[:])
                nc.vector.tensor_add(yt[:], yt[:], r0[:])
                nc.vector.tensor_add(yt[:], yt[:], r1[:])
                nc.sync.dma_start(out=out[row0:row0 + 128, :], in_=yt[:])

    nc.compile()
    _nc_cache[key] = nc
    return nc


def _route(x, gate_w, gate_b):
    """Host gate: softmax top-2 (float64 for stable ordering)."""
    logits = (x.astype(np.float64) @ gate_w.astype(np.float64)
              + gate_b.astype(np.float64))
    m = logits.max(axis=-1, keepdims=True)
    p = np.exp(logits - m)
    p /= p.sum(axis=-1, keepdims=True)
    order = np.argsort(-p, axis=-1)
    idx = order[:, :TOPK]                      # [T, 2]
    wts = np.take_along_axis(p, idx, axis=-1)  # [T, 2]
    return idx, wts.astype(np.float32)


def kernel(x, gate_w, gate_b, shared_w1, shared_b1, shared_w2, shared_b2,
           routed_w1, routed_b1, routed_w2, routed_b2):
    x = np.asarray(x, dtype=np.float32)
    topk_idx, topk_w = _route(x, np.asarray(gate_w), np.asarray(gate_b))

    owner = np.arange(T) // TO                 # owning core per token

    # per-expert dispatch lists, ordered by (owner, token)
    tok_lists, wt_lists = [], []
    for e in range(E):
        sel = (topk_idx == e)                  # [T, 2]
        tsel = np.nonzero(sel.any(axis=1))[0]  # ascending => owner-sorted
        k_of = sel[tsel, 1].astype(np.int64)   # slot (experts distinct)
        w_of = topk_w[tsel, :][np.arange(len(tsel)), k_of]
        tok_lists.append(tsel)
        wt_lists.append(w_of)

    c_max = max(len(tl) for tl in tok_lists)
    c_cap = max(C_CAP_DEFAULT, -(-c_max // 256) * 256)
    c_half = c_cap // 2

    # per-(expert, owner, half) positions + max chunk occupancy
    pair_max = 0
    pos_all, half_all = [], []
    for e in range(E):
        toks = tok_lists[e]
        own = owner[toks]
        cols = np.arange(len(toks))
        hh = (cols // c_half).astype(np.int64)
        pos = np.zeros(len(toks), np.int64)
        for o in range(NCORES):
            for h in range(2):
                mask = (own == o) & (hh == h)
                n = int(mask.sum())
                pos[mask] = np.arange(n)
                pair_max = max(pair_max, n)
        pos_all.append(pos)
        half_all.append(hh)
    p_cap = max(P_CAP_DEFAULT, -(-pair_max // 64) * 64)
    rows_h = NCORES * p_cap

    nc = _build(c_cap, p_cap)

    # host-side layouts (fp16 compute dtype)
    w1r = np.asarray(routed_w1, np.float16)              # [E, D, 2F]
    w2r = np.asarray(routed_w2, np.float16)              # [E, F, D]
    sw1r = np.asarray(shared_w1, np.float16)[0]          # [D, 2F]
    sw2r = np.asarray(shared_w2, np.float16)[0]          # [F, D]
    xr = x.astype(np.float16)                            # [T, D]

    def tile_w1(w):                            # [D,2F] -> [64, 8, 128, 128]
        return np.ascontiguousarray(
            w.reshape(D // 128, 128, 2 * F // 128, 128).transpose(2, 0, 1, 3))

    def col_bias(b):                           # [2F] -> [128, 64]
        return np.ascontiguousarray(
            np.asarray(b, np.float32).reshape(2 * F // 128, 128).T)

    sw1_t = tile_w1(sw1r)
    sb1_t = col_bias(np.asarray(shared_b1)[0])

    # absolute a2a_out row for each (token, slot)
    slot_rows = np.zeros((T, TOPK), np.int64)
    for e in range(E):
        toks = tok_lists[e]
        sel = (topk_idx[toks] == e)
        k_of = sel[:, 1].astype(np.int64)
        rows = half_all[e] * rows_h + e * p_cap + pos_all[e]
        slot_rows[toks, k_of] = rows

    in_maps = []
    for c in range(NCORES):
        e = c
        toks = tok_lists[e]
        wts = wt_lists[e]
        ce = len(toks)

        xg_a = np.zeros((D // 128, 128, c_cap), np.float16)
        xg_a[:, :, :ce] = xr[toks].T.reshape(D // 128, 128, ce)

        cw_a = np.zeros((c_cap, 1), np.float32)
        cw_a[:ce, 0] = wts

        scat_a = np.full((c_cap, 1), 2**31 - 1, np.int32)
        scat_a[:ce, 0] = (owner[toks] * p_cap + pos_all[e]).astype(np.int32)

        xs_a = np.ascontiguousarray(
            xr[c * TO:(c + 1) * TO].T.reshape(D // 128, 128, TO))

        g0 = slot_rows[c * TO:(c + 1) * TO, 0].astype(np.int32).reshape(TO, 1)
        g1 = slot_rows[c * TO:(c + 1) * TO, 1].astype(np.int32).reshape(TO, 1)

        in_maps.append({
            "xg": xg_a, "xs": xs_a,
            "w1": tile_w1(w1r[e]), "w2": np.ascontiguousarray(w2r[e]),
            "sw1": sw1_t, "sw2": sw2r,
            "b1": col_bias(np.asarray(routed_b1)[e]),
            "sb1": sb1_t,
            "b2": np.asarray(routed_b2, np.float32)[e].reshape(1, D).copy(),
            "sb2": np.asarray(shared_b2, np.float32)[0].reshape(1, D).copy(),
            "cw": cw_a, "scat": scat_a, "g0i": g0, "g1i": g1,
        })

    res = run_bass_kernel_spmd(nc, in_maps, list(range(NCORES)))
    return np.concatenate([res.results[c]["out"] for c in range(NCORES)],
                          axis=0)

